# revision 45
# baseline (speedup 1.0000x reference)
"""Trainium2 Bass kernel for nn_AttentionMatrix.

Computes, for mat_0:[B,N,H], mat_1:[B,M,H], w:[3H], bias:[1]:
    out[b,n,m] = sum_h mat_0[b,n,h]*w2[h]*mat_1[b,m,h] + s0[b,n] + s1[b,m] + C
with s0 = mat_0@w0, s1 = mat_1@w1, C = bias[0].

Strategy: data-parallel over batch across 8 NeuronCores (2 batches/core).

Math: the 68.7 GFLOP einsum runs on the PE array in fp8 DoubleRow perf
mode (2 contraction planes per instruction, 0.5 cycles/row — 4x the f32
rate) with a residual-correction scheme, 4 matmuls per 512-chunk:
    A@B ~= A8@B8 (both pairs) + A8@Br8 (pair0) + Ar5@B8 (pair0)
where A8,B8 = e4m3(aw), e4m3(b); Br8 = e4m3(b - B8) corrects the B-side
quantization error and Ar5 = e5m2(aw - A8) the A-side (e5m2's wide
exponent range holds the tiny residuals). The h axis is pre-sorted by
descending |w2| so pair0 (the residual-corrected half) carries the
high-magnitude rows; the uncorrected error lives only on low-signal
rows. Measured end-to-end rel_l2 vs f32 reference: 1.0e-2 (gate 2e-2).

Host-side prep (layout/quantization only + the 0.1%-of-FLOPs rank-1
vectors): pre-transposed, pre-scaled (w2 folded into A), pair-packed
[pair, p, plane, col] fp8 operands; s0/s1 rows for the epilogue.

Device: psum[128n, 1024m] accumulates 6 DoubleRow matmuls per 512-chunk;
fused epilogue (psum + s0_col + s1_row -> bf16) alternates between the
DVE and Pool engines; bf16 stores ride SP/ACT HWDGE queues. Tensor-engine
p-state is pre-warmed with throwaway matmuls during the load head; the
final tile drains in shrinking chunks with the rank-1 terms folded into
K=1 matmuls so the very last store rides the idle Pool SWDGE queue.
Host upconverts the bf16 output to f32.
"""

import numpy as np

import concourse.bacc as bacc
import concourse.bass as bass
import concourse.mybir as mybir
from concourse.tile import TileContext

F32 = mybir.dt.float32
BF16 = mybir.dt.bfloat16
E4 = mybir.dt.float8e4
E5 = mybir.dt.float8e5
ADD = mybir.AluOpType.add
DR = mybir.MatmulPerfMode.DoubleRow

P = 128

# Problem dims (hardcoded per contract)
B, N, M, H = 16, 2048, 2048, 512
N_CORES = 8
BPC = B // N_CORES  # batches per core


def build_program(bpc=BPC, n=N, m=M, h=H):
    nt = n // P        # n-tiles (16)
    hw = n // 2        # half width (1024)
    npair = h // 256   # DoubleRow plane-pairs (2)

    nc = bacc.Bacc("TRN2", target_bir_lowering=False, debug=False)
    # pair-packed fp8 operands: [bpc, pair, p, plane, cols]
    a8d = nc.dram_tensor("a8", [bpc, npair, P, 2, n], E4, kind="ExternalInput").ap()
    ar5d = nc.dram_tensor("ar5", [bpc, npair, P, 2, n], E5, kind="ExternalInput").ap()
    b8d = nc.dram_tensor("b8", [bpc, npair, P, 2, m], E4, kind="ExternalInput").ap()
    br8d = nc.dram_tensor("br8", [bpc, npair, P, 2, m], E4, kind="ExternalInput").ap()
    s0a = nc.dram_tensor("s0a", [P, bpc * nt], F32, kind="ExternalInput").ap()
    s0rb = nc.dram_tensor("s0rb", [1, bpc * n], BF16, kind="ExternalInput").ap()
    s1rb = nc.dram_tensor("s1rb", [1, bpc * m], BF16, kind="ExternalInput").ap()
    out = nc.dram_tensor("out", [bpc, n, m], BF16, kind="ExternalOutput").ap()

    with TileContext(nc) as tc:
        with (
            tc.tile_pool(name="const", bufs=1) as cpool,
            tc.tile_pool(name="ops", bufs=2) as ops,
            tc.tile_pool(name="vecs", bufs=1) as vpool,
            tc.tile_pool(name="ob", bufs=12) as obpool,
            tc.tile_pool(name="mpsum", bufs=4, space="PSUM") as mpsum,
        ):
            # PE p-state warm-up: the tensor engine ramps 0.65->1.2->2.4 GHz
            # over ~3us of continuous execution. Run throwaway matmuls on a
            # zeroed tile while the first operand stripes stream in.
            warm = cpool.tile([P, 512], BF16)
            nc.vector.memset(warm, 0.0)
            ones = cpool.tile([1, 512], BF16)
            nc.gpsimd.memset(ones, 1.0)
            wp = mpsum.tile([P, hw], F32, tag="mm", name="wp")
            for _ in range(6):
                nc.tensor.matmul(
                    wp[:, :512], lhsT=warm[:, :P], rhs=warm,
                    start=True, stop=True,
                )
            for _ in range(40):
                nc.tensor.matmul(
                    wp[:, :16], lhsT=warm[:, :P], rhs=warm[:, :16],
                    start=True, stop=True,
                )

            def emit_vecs():
                # all vector loads ride the Pool SWDGE queue: zero pressure
                # on the shared HWDGE issue pipeline feeding stripes/stores
                s0all = cpool.tile([P, bpc * nt], F32)
                nc.gpsimd.dma_start(out=s0all, in_=s0a)
                s1row = cpool.tile([1, bpc * m], BF16)
                nc.gpsimd.dma_start(out=s1row, in_=s1rb)
                s1bs = []
                for bi in range(bpc):
                    s1b = vpool.tile([P, m], BF16, tag=f"s1b{bi}",
                                     name=f"s1b{bi}")
                    nc.gpsimd.partition_broadcast(
                        s1b, s1row[0:1, bi * m:(bi + 1) * m]
                    )
                    s1bs.append(s1b)
                s0row = cpool.tile([1, bpc * n], BF16)
                nc.gpsimd.dma_start(out=s0row, in_=s0rb)
                vpool.rank1 = (s0row, s1row)
                return s0all, s1bs

            def emit_loads(bi, first=False, only=None):
                # tiles[name][(q, j)] -> [P, 2*hw] tile viewed as [P,2,hw]
                tiles = {nm: {} for nm in ("a8", "ar5", "b8", "br8")}
                srcs = {"a8": a8d, "ar5": ar5d, "b8": b8d, "br8": br8d}
                dts = {"a8": E4, "ar5": E5, "b8": E4, "br8": E4}

                def load(nm, q, j):
                    t = ops.tile([P, 2 * hw], dts[nm], tag=f"{nm}q{q}h{j}",
                                 name=f"{nm}q{q}h{j}")
                    nc.sync.dma_start(
                        out=t.rearrange("p (two f) -> p two f", two=2),
                        in_=srcs[nm][bi, q, :, :, j * hw:(j + 1) * hw],
                    )
                    tiles[nm][(q, j)] = t

                vecs = emit_vecs() if first else None
                # supply order matched to the 6-term accumulation order of
                # the first in-flight tile group (h0), then rhs h1, lhs h1
                if only in (None, 0):
                    load("b8", 0, 0); load("a8", 0, 0)
                    load("b8", 1, 0); load("a8", 1, 0)
                    load("br8", 0, 0); load("ar5", 0, 0)
                if only in (None, 1):
                    load("b8", 0, 1); load("b8", 1, 1)
                    load("br8", 0, 1)
                    load("a8", 0, 1); load("a8", 1, 1)
                    load("ar5", 0, 1)
                if vecs is not None:
                    return tiles, vecs
                return tiles

            def two(t):
                return t.rearrange("p (two f) -> p two f", two=2)

            def lhsT_ap(tiles, nm, q, t):
                j, col = t // 8, (t % 8) * P
                return two(tiles[nm][(q, j)])[:, :, col:col + P]

            def rhs_ap(tiles, nm, q, c0, cw):
                # c0 absolute column, cw width (within one half)
                j = c0 // hw
                col = c0 - j * hw
                return two(tiles[nm][(q, j)])[:, :, col:col + cw]

            TERMS = (("a8", "b8", 0), ("a8", "b8", 1),
                     ("a8", "br8", 0), ("ar5", "b8", 0))

            def emit_chunk(tiles, mp, mo, t, c0, cw, stop=True):
                for ti, (lnm, rnm, q) in enumerate(TERMS):
                    nc.tensor.matmul(
                        mp[:, mo:mo + cw],
                        lhsT=lhsT_ap(tiles, lnm, q, t),
                        rhs=rhs_ap(tiles, rnm, q, c0, cw),
                        start=(ti == 0),
                        stop=stop and (ti == len(TERMS) - 1),
                        perf_mode=DR,
                    )

            def emit_epilogue(bi, mp, t, half, oi, s0c, s1b,
                              tail=False):
                ob = obpool.tile([P, hw], BF16, tag="ob", name="ob")
                if oi % 3 != 0 or tail:
                    # two-stage eviction: ACT folds +s0 while copying psum
                    # to bf16; DVE then adds the s1 row in its all-2-byte
                    # 2x mode — halves DVE pressure (and frees DVE early
                    # for the drain chunks at the tail)
                    tmp = obpool.tile([P, hw], BF16, tag="tmp", name="tmp")
                    nc.scalar.add(tmp, mp, s0c[:, t:t + 1])
                    nc.vector.tensor_add(
                        ob, tmp, s1b[:, half * hw:(half + 1) * hw]
                    )
                else:
                    nc.vector.scalar_tensor_tensor(
                        out=ob,
                        in0=mp,
                        scalar=s0c[:, t:t + 1],
                        in1=s1b[:, half * hw:(half + 1) * hw],
                        op0=ADD,
                        op1=ADD,
                    )
                nc.sync.dma_start(
                    out=out[bi, t * P:(t + 1) * P,
                            half * hw:(half + 1) * hw],
                    in_=ob,
                )

            def emit_mains(bi, tiles, s0all, s1bs, last=False,
                           emit_mid=None):
                s0c = s0all[:, bi * nt:(bi + 1) * nt]
                s1b = s1bs[bi]
                order = []
                for g in range(0, nt, 4):
                    order += [(t, 0) for t in range(g, g + 4)]
                    order += [(t, 1) for t in range(g, g + 4)]
                mid_result = None
                if bi == 0:
                    # pipeline head: emit the first two psum waves
                    # level-major (all tiles' term j before term j+1) so PE
                    # demand tracks the DMA supply order tile-by-tile
                    for wave in (order[0:4], order[4:8]):
                        mps = []
                        for (t, half) in wave:
                            mps.append(mpsum.tile([P, hw], F32, tag="mm",
                                                  name="mp"))
                        for ti, (lnm, rnm, q) in enumerate(TERMS):
                            for wi, (t, half) in enumerate(wave):
                                for cc in range(2):
                                    nc.tensor.matmul(
                                        mps[wi][:, cc * 512:(cc + 1) * 512],
                                        lhsT=lhsT_ap(tiles, lnm, q, t),
                                        rhs=rhs_ap(tiles, rnm, q,
                                                   half * hw + cc * 512,
                                                   512),
                                        start=(ti == 0),
                                        stop=(ti == len(TERMS) - 1),
                                        perf_mode=DR,
                                    )
                        for wi, (t, half) in enumerate(wave):
                            oi = order.index((t, half))
                            emit_epilogue(bi, mps[wi], t, half, oi,
                                          s0c, s1b)
                    order = order[8:]
                for oi2, (t, half) in enumerate(order):
                    oi = oi2 + (8 if bi == 0 else 0)
                    if emit_mid is not None and oi in (16, 24):
                        r = emit_mid(0 if oi == 16 else 1)
                        if oi == 16:
                            mid_result = r
                    if last and t == nt - 1 and half == 1:
                        # final half-tile: shrinking chunk drain; the last
                        # (smallest) chunk's store rides the idle Pool
                        # SWDGE queue for the shortest tail chain
                        s0row, s1row = vpool.rank1
                        drains = ((1024, 512, nc.sync, False),
                                  (1536, 384, nc.scalar, False),
                                  (1920, 128, nc.gpsimd, True))
                        for c0, cw, eng, fold in drains:
                            mp = mpsum.tile([P, hw], F32, tag="mm",
                                            name="mp")
                            emit_chunk(tiles, mp, 0, t, c0, cw,
                                       stop=not fold)
                            ob = obpool.tile([P, 512], BF16, tag="obl",
                                             name="obl")
                            if fold:
                                # rank-1 epilogue folded into K=1 matmuls;
                                # eviction is a plain copy on the idle ACT
                                nc.tensor.matmul(
                                    mp[:, :cw],
                                    lhsT=s0row[0:1, bi * n + t * P:
                                               bi * n + (t + 1) * P],
                                    rhs=ones[0:1, :cw],
                                    start=False, stop=False,
                                )
                                nc.tensor.matmul(
                                    mp[:, :cw],
                                    lhsT=ones[0:1, :P],
                                    rhs=s1row[0:1, bi * m + c0:
                                              bi * m + c0 + cw],
                                    start=False, stop=True,
                                )
                                nc.scalar.copy(
                                    out=ob[:, :cw], in_=mp[:, :cw]
                                )
                            else:
                                nc.vector.scalar_tensor_tensor(
                                    out=ob[:, :cw],
                                    in0=mp[:, :cw],
                                    scalar=s0c[:, t:t + 1],
                                    in1=s1b[:, c0:c0 + cw],
                                    op0=ADD,
                                    op1=ADD,
                                )
                            eng.dma_start(
                                out=out[bi, t * P:(t + 1) * P, c0:c0 + cw],
                                in_=ob[:, :cw],
                            )
                        continue
                    mp = mpsum.tile([P, hw], F32, tag="mm", name="mp")
                    for cc in range(2):
                        emit_chunk(tiles, mp, cc * 512, t,
                                   half * hw + cc * 512, 512)
                    emit_epilogue(bi, mp, t, half, oi, s0c, s1b,
                                  tail=last and oi >= 60)
                return mid_result

            tiles0, (s0all, s1bs) = emit_loads(0, first=True)
            la = tiles0
            for bi in range(1, bpc):
                part = {}

                def mid(which, bi=bi, part=part):
                    t = emit_loads(bi, only=which)
                    if which == 0:
                        part.update(t)
                    else:
                        for k, v in t.items():
                            part[k].update(v)
                    return part
                emit_mains(bi - 1, la, s0all, s1bs, emit_mid=mid)
                la = part
            emit_mains(bpc - 1, la, s0all, s1bs, last=True)
    nc.compile()
    return nc


_CACHE = {}


def _get_program():
    if "nc" not in _CACHE:
        _CACHE["nc"] = build_program()
    return _CACHE["nc"]


def _pack_pairs(x):
    """[B, H, W] -> [B, pair, p, plane, W] with h = pair*256 + plane*128 + p."""
    Bn, Hh, W = x.shape
    return np.ascontiguousarray(
        x.reshape(Bn, Hh // 256, 2, P, W).transpose(0, 1, 3, 2, 4)
    )


def make_in_maps(inputs, bpc=BPC, n_cores=N_CORES, n=N, m=M, h=H):
    import ml_dtypes

    bf16 = ml_dtypes.bfloat16
    e4 = ml_dtypes.float8_e4m3fn
    e5 = ml_dtypes.float8_e5m2
    mat_0 = np.asarray(inputs["mat_0"], dtype=np.float32)
    mat_1 = np.asarray(inputs["mat_1"], dtype=np.float32)
    w = np.asarray(inputs["w"], dtype=np.float32)
    bias = np.asarray(inputs["bias"], dtype=np.float32)
    w0, w1, w2 = w[:h], w[h:2 * h], w[2 * h:]
    nt = n // P
    # host-side rank-1 epilogue vectors
    s0 = mat_0 @ w0                      # [B, n]
    s1 = mat_1 @ w1 + bias[0]            # [B, m]
    # pre-transposed operands and fp8 residual decomposition. The h axis
    # is permuted by descending |w2| so the plane-pair-0 residual terms
    # correct the high-magnitude half of the contraction (the uncorrected
    # quantization error then lives only on low-signal rows).
    perm = np.argsort(-np.abs(w2))
    awt = np.ascontiguousarray(
        (mat_0 * w2).transpose(0, 2, 1)[:, perm, :])             # [B, h, n]
    bt = np.ascontiguousarray(
        mat_1.transpose(0, 2, 1)[:, perm, :])                    # [B, h, m]
    a8 = awt.astype(e4)
    ar5 = (awt - a8.astype(np.float32)).astype(e5)
    b8 = bt.astype(e4)
    br8 = (bt - b8.astype(np.float32)).astype(e4)
    a8 = _pack_pairs(a8)
    ar5 = _pack_pairs(ar5)
    b8 = _pack_pairs(b8)
    br8 = _pack_pairs(br8)
    s0t = np.ascontiguousarray(
        s0.reshape(-1, nt, P).transpose(0, 2, 1)              # [B, P, nt]
    )
    in_maps = []
    for c in range(n_cores):
        sl = slice(c * bpc, (c + 1) * bpc)
        s0a = np.ascontiguousarray(
            s0t[sl].transpose(1, 0, 2).reshape(P, bpc * nt)
        )
        in_maps.append(
            {
                "a8": a8[sl],
                "ar5": ar5[sl],
                "b8": b8[sl],
                "br8": br8[sl],
                "s0a": s0a,
                "s0rb": np.ascontiguousarray(
                    s0[sl].reshape(1, bpc * n)).astype(bf16),
                "s1rb": np.ascontiguousarray(
                    s1[sl].reshape(1, bpc * m)).astype(bf16),
            }
        )
    return in_maps


def kernel(**inputs) -> np.ndarray:
    from concourse import bass_utils

    nc = _get_program()
    res = bass_utils.run_bass_kernel_spmd(
        nc, make_in_maps(inputs), core_ids=list(range(N_CORES))
    )
    return np.concatenate(
        [np.asarray(res.results[c]["out"]).astype(np.float32)
         for c in range(N_CORES)],
        axis=0,
    )
